# revision 11
# baseline (speedup 1.0000x reference)
"""Biased multi-head self-attention (B=4, N=1024, H=1024, 16 heads) on 8
Trainium2 NeuronCores.

Sharding: data-parallel over batch (4) x tensor-parallel over head-groups
(2 groups of 8 heads) = 8 cores. Core c handles batch c//2, head-group c%2.
Each core computes QKV projections for its 512 feature columns, biased
softmax attention for its 8 heads, and a partial output projection
(contracting its 512 feature rows of Wp). The two head-groups' partial
projections per batch are summed on the host (the "projection all-reduce"),
which also adds the output bias bp.

Device dataflow (per core); all matmuls run as fp32r (full-rate PE mode):
  - xT [h, n] (host-transposed x) and weights [h, d] feed the PE directly:
    qT/kT land in [d, n] layout, v in [n, d] layout.
  - The attention-score scale 1/sqrt(64) is folded into Wq/bq on the host.
  - Scores are computed TRANSPOSED, sT[m, n] (lhsT=kT, rhs=qT, K=64),
    because the PV matmul contracts over m, which must live on partitions;
    attn_bias is pre-transposed on the host for the same reason.
  - eT = exp(sT + biasT): DVE add (psum + sbuf), ACT exp in place.
  - v is stored with an interleaved all-ones column per head (via an
    augmented Wv/bv), so each head's PV matmul yields [65, n]: rows 0..63
    are unnormalized feats^T, row 64 is the softmax denominator s[n].
  - Normalization multiplies by 1/s broadcast across partitions via a K=1
    ones matmul.
  - out_partial [n, 1024] = featsT.T @ Wp_slice.
"""

import sys

for _p in ("/opt/trn_rl_repo", "/opt/pypackages"):
    if _p not in sys.path:
        sys.path.append(_p)

import numpy as np

import concourse.bass as bass
import concourse.bacc as bacc
import concourse.mybir as mybir
import concourse.tile as tile
from concourse.bass_utils import run_bass_kernel_spmd

P = 128
N = 1024          # sequence length
H = 1024          # model dim
B = 4
NH = 16
HS = 64
G = 2             # head groups (tensor parallel)
HL = NH // G      # heads per core = 8
DLOC = H // G     # feature cols per core = 512
DAUG = HL * (HS + 1)   # 520: v with interleaved ones column
HC = H // P       # 8 contraction chunks over model dim
DC = DLOC // P    # 4 chunks over local feature dim
NB = N // 512     # 2 moving-dim blocks
NT = N // P       # 8 n tiles
MC = N // P       # 8 m chunks
SCALE = 1.0 / np.sqrt(HS)

F32 = mybir.dt.float32
F32R = mybir.dt.float32r
Act = mybir.ActivationFunctionType

_PROG = None


def _emit(nc, tc, io):
    xT, biasT, wq, wk, wv, wp, bq, bk, bv, ones, ident_d, out = io

    import contextlib

    with contextlib.ExitStack() as ctx:
        consts = ctx.enter_context(tc.tile_pool(name="consts", bufs=1))
        qkv = ctx.enter_context(tc.tile_pool(name="qkv", bufs=1))
        opool = ctx.enter_context(tc.tile_pool(name="opool", bufs=2))
        small = ctx.enter_context(tc.tile_pool(name="small", bufs=3))
        sps = ctx.enter_context(tc.tile_pool(name="sps", bufs=2, space="PSUM"))
        fps = ctx.enter_context(tc.tile_pool(name="fps", bufs=4, space="PSUM"))
        stage1_ctx = contextlib.ExitStack()
        stage1 = stage1_ctx.enter_context(tc.tile_pool(name="stage1", bufs=1))
        wpool = stage1_ctx.enter_context(tc.tile_pool(name="wpool", bufs=2))

        ones_t = consts.tile([1, P], F32R)
        nc.gpsimd.dma_start(out=ones_t, in_=ones)
        inv_scr = nc.dram_tensor("inv_scr", [4, 512], F32).ap()
        ident = consts.tile([P, P], F32R)
        nc.sync.dma_start(out=ident, in_=ident_d)

        xT_sb = stage1.tile([P, HC, N], F32R)
        nc.sync.dma_start(out=xT_sb, in_=xT.rearrange("(c p) n -> p c n", p=P))

        wp_sb = consts.tile([P, DC, H], F32R)
        nc.sync.dma_start(out=wp_sb, in_=wp.rearrange("(c p) o -> p c o", p=P))
        bq_sb = consts.tile([P, DC], F32)
        nc.gpsimd.dma_start(out=bq_sb, in_=bq)
        bk_sb = consts.tile([P, DC], F32)
        nc.gpsimd.dma_start(out=bk_sb, in_=bk)
        bv_sb = consts.tile([1, DAUG], F32R)
        nc.gpsimd.dma_start(out=bv_sb, in_=bv)

        qT_sb = qkv.tile([P, DC, N], F32R)
        kT_sb = qkv.tile([P, DC, N], F32R)
        v_sb = qkv.tile([P, MC, DAUG], F32R)
        featsT_sb = qkv.tile([P, DC, N], F32R)

        # ---- QKV projections ----
        for w_dram, b_sb, dst in ((wq, bq_sb, qT_sb), (wk, bk_sb, kT_sb)):
            w_t = wpool.tile([P, HC, DAUG], F32R, tag="w")
            nc.sync.dma_start(
                out=w_t[:, :, :DLOC], in_=w_dram.rearrange("(c p) d -> p c d", p=P)
            )
            for dc in range(DC):
                ps = sps.tile([P, N], F32, tag="sps")
                for nb in range(NB):
                    for hc in range(HC):
                        nc.tensor.matmul(
                            ps[:, nb * 512 : (nb + 1) * 512],
                            (w_t[:, hc, dc * P : (dc + 1) * P]),
                            (xT_sb[:, hc, nb * 512 : (nb + 1) * 512]),
                            start=(hc == 0),
                            stop=(hc == HC - 1),
                        )
                nc.scalar.activation(
                    out=dst[:, dc, :],
                    in_=ps,
                    func=Act.Identity,
                    bias=b_sb[:, dc : dc + 1],
                )

        wv_t = wpool.tile([P, HC, DAUG], F32R, tag="w")
        nc.sync.dma_start(out=wv_t, in_=wv.rearrange("(c p) d -> p c d", p=P))
        HF = DAUG // 2  # 260
        for nt in range(NT):
            ps = sps.tile([P, N], F32, tag="sps")
            for half in range(2):
                pv = ps[:, half * 512 : half * 512 + HF]
                for hc in range(HC):
                    nc.tensor.matmul(
                        pv,
                        (xT_sb[:, hc, nt * P : (nt + 1) * P]),
                        (wv_t[:, hc, half * HF : (half + 1) * HF]),
                        start=(hc == 0),
                        stop=False,
                    )
                # bias row: v[n, :] += bv (also writes the ones columns)
                nc.tensor.matmul(
                    pv,
                    (ones_t[:1, :P]),
                    (bv_sb[:1, half * HF : (half + 1) * HF]),
                    start=False,
                    stop=True,
                )
            nc.scalar.copy(
                v_sb[:, nt, :].rearrange("p (h x) -> p h x", h=2),
                ps.rearrange("p (h x) -> p h x", h=2)[:, :, :HF],
            )

        stage1_ctx.close()
        bias_pool = ctx.enter_context(tc.tile_pool(name="bias", bufs=6))
        et_pool = ctx.enter_context(tc.tile_pool(name="et", bufs=3))

        # ---- attention, head pairs (row-packed scores) ----
        # Per (head, m-chunk): psum[m, n] = I @ biasT (start) + kT.T q (accum),
        # so the bias add runs on the PE and ACT exps straight out of PSUM.
        for hp in range(4):
            heads = (2 * hp, 2 * hp + 1)
            bias_t = {}

            def load_bias(h, j):
                bt = bias_pool.tile([P, 2, N], F32R, tag="bias", name=f"bt{h}_{j}")
                nc.sync.dma_start(
                    out=bt,
                    in_=biasT[h].rearrange("(c p) n -> p c n", p=P)[
                        :, 2 * j : 2 * j + 2
                    ],
                )
                bias_t[h] = bt

            f_ps = {
                (h, nb): fps.tile([HS + 1, 512], F32, tag="fps", name=f"fps{h}_{nb}")
                for h in heads
                for nb in range(NB)
            }
            for mc in range(MC):
                if mc % 2 == 0:
                    for h in heads:
                        load_bias(h, mc // 2)
                sp = {}
                et = {}
                for h in heads:
                    sp[h] = sps.tile([P, N], F32, tag="sps", name=f"sp{h}_{mc}")
                    et[h] = et_pool.tile([P, N], F32R, tag="et", name=f"et{h}_{mc}")
                    for nb in range(NB):
                        nc.tensor.matmul(
                            sp[h][:, nb * 512 : (nb + 1) * 512],
                            ident[:, :],
                            bias_t[h][:, mc % 2, nb * 512 : (nb + 1) * 512],
                            start=True,
                            stop=False,
                        )
                for nb in range(NB):
                    for h in heads:  # adjacent K=64 matmuls -> row-packed pair
                        dpo = (h % 2) * HS
                        nc.tensor.matmul(
                            sp[h][:, nb * 512 : (nb + 1) * 512],
                            kT_sb[dpo : dpo + HS, hp, mc * P : (mc + 1) * P],
                            qT_sb[dpo : dpo + HS, hp, nb * 512 : (nb + 1) * 512],
                            start=False,
                            stop=True,
                        )
                for h in heads:
                    nc.scalar.activation(out=et[h], in_=sp[h], func=Act.Exp)
                    for nb in range(NB):
                        nc.tensor.matmul(
                            f_ps[(h, nb)],
                            v_sb[:, mc, (HS + 1) * h : (HS + 1) * (h + 1)],
                            et[h][:, nb * 512 : (nb + 1) * 512],
                            start=(mc == 0),
                            stop=(mc == MC - 1),
                        )
            # normalize by the softmax denominator (psum row 64) and store
            for h in heads:
                po, ddc = HS * (h % 2), h // 2
                for nb in range(NB):
                    inv_s = small.tile([1, 512], F32, tag="inv", name=f"inv{h}{nb}")
                    nc.vector.reciprocal(inv_s, f_ps[(h, nb)][HS : HS + 1, :])
                    scr = inv_scr[2 * (h % 2) + nb : 2 * (h % 2) + nb + 1, :]
                    nc.gpsimd.dma_start(out=scr, in_=inv_s)
                    b_sb = small.tile([HS, 512], F32, tag="bcast", name=f"bc{h}{nb}")
                    nc.gpsimd.dma_start(out=b_sb, in_=scr.to_broadcast((HS, 512)))
                    nc.vector.tensor_mul(
                        out=featsT_sb[po : po + HS, ddc, nb * 512 : (nb + 1) * 512],
                        in0=f_ps[(h, nb)][:HS, :],
                        in1=b_sb,
                    )

        # ---- output projection (partial: contracts this core's 512 rows) ----
        for nt in range(NT):
            ps = sps.tile([P, N], F32, tag="sps")
            for cb in range(NB):
                for dc in range(DC):
                    nc.tensor.matmul(
                        ps[:, cb * 512 : (cb + 1) * 512],
                        (featsT_sb[:, dc, nt * P : (nt + 1) * P]),
                        (wp_sb[:, dc, cb * 512 : (cb + 1) * 512]),
                        start=(dc == 0),
                        stop=(dc == DC - 1),
                    )
            o_t = opool.tile([P, N], F32, tag="o")
            nc.scalar.copy(o_t, ps)
            nc.sync.dma_start(out=out[nt * P : (nt + 1) * P, :], in_=o_t)


def build_program():
    nc = bacc.Bacc("TRN2", target_bir_lowering=False, debug=False, num_devices=8)
    xT = nc.dram_tensor("xT", [H, N], F32R, kind="ExternalInput").ap()
    biasT = nc.dram_tensor("biasT", [HL, N, N], F32R, kind="ExternalInput").ap()
    wq = nc.dram_tensor("wq", [H, DLOC], F32R, kind="ExternalInput").ap()
    wk = nc.dram_tensor("wk", [H, DLOC], F32R, kind="ExternalInput").ap()
    wv = nc.dram_tensor("wv", [H, DAUG], F32R, kind="ExternalInput").ap()
    wp = nc.dram_tensor("wp", [DLOC, H], F32R, kind="ExternalInput").ap()
    bq = nc.dram_tensor("bq", [P, DC], F32, kind="ExternalInput").ap()
    bk = nc.dram_tensor("bk", [P, DC], F32, kind="ExternalInput").ap()
    bv = nc.dram_tensor("bv", [1, DAUG], F32R, kind="ExternalInput").ap()
    ones = nc.dram_tensor("ones", [1, P], F32R, kind="ExternalInput").ap()
    ident_d = nc.dram_tensor("ident", [P, P], F32R, kind="ExternalInput").ap()
    out = nc.dram_tensor("out", [N, H], F32, kind="ExternalOutput").ap()
    with tile.TileContext(nc) as tc:
        _emit(nc, tc, (xT, biasT, wq, wk, wv, wp, bq, bk, bv, ones, ident_d, out))
    nc.compile()
    return nc


def get_program():
    global _PROG
    if _PROG is None:
        _PROG = build_program()
    return _PROG


def make_in_maps(x, attn_bias, Wq, bq, Wk, bk, Wv, bv, Wp):
    """Host-side sharding: slice/transpose/augment per-core inputs."""
    f = np.float32
    x = np.asarray(x, f)
    attn_bias = np.asarray(attn_bias, f)
    wq_s = np.asarray(Wq, f) * f(SCALE)
    bq_s = np.asarray(bq, f) * f(SCALE)
    Wk, bk = np.asarray(Wk, f), np.asarray(bk, f)
    Wv, bv = np.asarray(Wv, f), np.asarray(bv, f)
    Wp = np.asarray(Wp, f)

    xTs = [np.ascontiguousarray(x[b].T) for b in range(B)]
    in_maps = []
    for c in range(8):
        b, g = divmod(c, 2)
        dsl = slice(DLOC * g, DLOC * (g + 1))
        wv_aug = np.zeros((H, DAUG), f)
        bv_aug = np.zeros((1, DAUG), f)
        for hl in range(HL):
            src = slice(DLOC * g + HS * hl, DLOC * g + HS * (hl + 1))
            dst = slice((HS + 1) * hl, (HS + 1) * hl + HS)
            wv_aug[:, dst] = Wv[:, src]
            bv_aug[0, dst] = bv[src]
            bv_aug[0, (HS + 1) * hl + HS] = 1.0
        in_maps.append(
            {
                "xT": xTs[b],
                "biasT": np.ascontiguousarray(
                    attn_bias[b, HL * g : HL * (g + 1)].transpose(0, 2, 1)
                ),
                "wq": np.ascontiguousarray(wq_s[:, dsl]),
                "wk": np.ascontiguousarray(Wk[:, dsl]),
                "wv": wv_aug,
                "wp": np.ascontiguousarray(Wp[dsl, :]),
                "bq": np.ascontiguousarray(bq_s[dsl].reshape(DC, P).T),
                "bk": np.ascontiguousarray(bk[dsl].reshape(DC, P).T),
                "bv": bv_aug,
                "ones": np.ones((1, P), f),
                "ident": np.eye(P, dtype=f),
            }
        )
    return in_maps


def _ensure_ntff_hook():
    """Register the axon NTFF profile hook if the image's antenv lacks it."""
    try:
        from antenv.axon_hooks import get_axon_ntff_profile_hook  # noqa: F401

        return
    except ImportError:
        pass
    import types

    import antenv
    from trn_agent_boot.trn_boot import _ntff_profile_via_ctypes

    mod = types.ModuleType("antenv.axon_hooks")
    box = {"h": None}
    mod.set_axon_ntff_profile_hook = lambda h: box.__setitem__("h", h)
    mod.get_axon_ntff_profile_hook = lambda: box["h"]
    sys.modules["antenv.axon_hooks"] = mod
    antenv.axon_hooks = mod
    hook = _ntff_profile_via_ctypes("/opt/axon/libaxon_pjrt.so")
    if hook is not None:
        mod.set_axon_ntff_profile_hook(hook)


def run_cores(in_maps, trace=False):
    nc = get_program()
    kwargs = {}
    if trace:
        _ensure_ntff_hook()
        kwargs = dict(trace=True, trace_cores=[0])
    return run_bass_kernel_spmd(nc, in_maps, core_ids=list(range(8)), **kwargs)


def kernel(x, attn_bias, Wq, bq, Wk, bk, Wv, bv, Wp, bp):
    in_maps = make_in_maps(x, attn_bias, Wq, bq, Wk, bk, Wv, bv, Wp)
    res = run_cores(in_maps)
    bp = np.asarray(bp, np.float32)
    out = np.empty((B, N, H), np.float32)
    for b in range(B):
        out[b] = res.results[2 * b]["out"] + res.results[2 * b + 1]["out"] + bp
    return out


# revision 13
# speedup vs baseline: 1.1948x; 1.1948x over previous
"""Biased multi-head self-attention (B=4, N=1024, H=1024, 16 heads) on 8
Trainium2 NeuronCores.

Sharding: data-parallel over batch (4) x tensor-parallel over head-groups
(2 groups of 8 heads) = 8 cores. Core c handles batch c//2, head-group c%2.
Each core computes QKV projections for its 512 feature columns, biased
softmax attention for its 8 heads, and a partial output projection
(contracting its 512 feature rows of Wp). The two head-groups' partial
projections per batch are summed on the host (the "projection all-reduce"),
which also adds the output bias bp.

Device dataflow (per core); all matmuls run as fp32r (full-rate PE mode):
  - xT [h, n] (host-transposed x) and weights [h, d] feed the PE directly:
    qT/kT land in [d, n] layout, v in [n, d] layout.
  - The attention-score scale 1/sqrt(64) is folded into Wq/bq on the host.
  - Scores are computed TRANSPOSED, sT[m, n] (lhsT=kT, rhs=qT, K=64),
    because the PV matmul contracts over m, which must live on partitions;
    attn_bias is pre-transposed on the host for the same reason.
  - eT = exp(sT + biasT): DVE add (psum + sbuf), ACT exp in place.
  - v is stored with an interleaved all-ones column per head (via an
    augmented Wv/bv), so each head's PV matmul yields [65, n]: rows 0..63
    are unnormalized feats^T, row 64 is the softmax denominator s[n].
  - Normalization multiplies by 1/s broadcast across partitions via a K=1
    ones matmul.
  - out_partial [n, 1024] = featsT.T @ Wp_slice.
"""

import sys

for _p in ("/opt/trn_rl_repo", "/opt/pypackages"):
    if _p not in sys.path:
        sys.path.append(_p)

import numpy as np

import concourse.bass as bass
import concourse.bacc as bacc
import concourse.mybir as mybir
import concourse.tile as tile
from concourse.bass_utils import run_bass_kernel_spmd

P = 128
N = 1024          # sequence length
H = 1024          # model dim
B = 4
NH = 16
HS = 64
G = 2             # head groups (tensor parallel)
HL = NH // G      # heads per core = 8
DLOC = H // G     # feature cols per core = 512
DAUG = HL * (HS + 1)   # 520: v with interleaved ones column
HC = H // P       # 8 contraction chunks over model dim
DC = DLOC // P    # 4 chunks over local feature dim
NB = N // 512     # 2 moving-dim blocks
NT = N // P       # 8 n tiles
MC = N // P       # 8 m chunks
SCALE = 1.0 / np.sqrt(HS)

F32 = mybir.dt.float32
F32R = mybir.dt.float32r
F16 = mybir.dt.float16
Act = mybir.ActivationFunctionType

_PROG = None


def _emit(nc, tc, io):
    xT, biasT, wq, wk, wv, wp, bq, bk, bv, ones, ident_d, out = io

    import contextlib

    with contextlib.ExitStack() as ctx:
        consts = ctx.enter_context(tc.tile_pool(name="consts", bufs=1))
        qkv = ctx.enter_context(tc.tile_pool(name="qkv", bufs=1))
        opool = ctx.enter_context(tc.tile_pool(name="opool", bufs=2))
        small = ctx.enter_context(tc.tile_pool(name="small", bufs=3))
        sps = ctx.enter_context(tc.tile_pool(name="sps", bufs=2, space="PSUM"))
        fps = ctx.enter_context(tc.tile_pool(name="fps", bufs=4, space="PSUM"))
        bias_pool = ctx.enter_context(tc.tile_pool(name="bias", bufs=6))
        et_pool = ctx.enter_context(tc.tile_pool(name="et", bufs=3))
        stage1_ctx = contextlib.ExitStack()
        stage1 = stage1_ctx.enter_context(tc.tile_pool(name="stage1", bufs=1))

        ones_t = consts.tile([1, P], F32R)
        nc.gpsimd.dma_start(out=ones_t, in_=ones)
        inv_scr = nc.dram_tensor("inv_scr", [4, 512], F32).ap()
        ident = consts.tile([P, P], F16)
        nc.sync.dma_start(out=ident, in_=ident_d)

        xT_r = xT.rearrange("(c p) n -> p c n", p=P)
        wq_r = wq.rearrange("(c p) d -> p c d", p=P)
        wk_r = wk.rearrange("(c p) d -> p c d", p=P)
        xT_t, wq_t, wk_t, wv_t = [], [], [], []
        for hc in range(HC):
            xt = stage1.tile([P, N], F32R, name=f"xt{hc}")
            nc.sync.dma_start(out=xt, in_=xT_r[:, hc])
            xT_t.append(xt)
            wt = stage1.tile([P, DLOC], F32R, name=f"wq{hc}")
            nc.sync.dma_start(out=wt, in_=wq_r[:, hc])
            wq_t.append(wt)
        for hc in range(HC):
            wt = stage1.tile([P, DLOC], F32R, name=f"wk{hc}")
            nc.sync.dma_start(out=wt, in_=wk_r[:, hc])
            wk_t.append(wt)
        wv_r = wv.rearrange("(c p) d -> p c d", p=P)
        for hc in range(HC):
            wt = stage1.tile([P, DAUG], F32R, name=f"wv{hc}")
            nc.sync.dma_start(out=wt, in_=wv_r[:, hc])
            wv_t.append(wt)

        wp_sb = consts.tile([P, DC, H], F32R)
        nc.sync.dma_start(out=wp_sb, in_=wp.rearrange("(c p) o -> p c o", p=P))
        bq_sb = consts.tile([P, DC], F32)
        nc.gpsimd.dma_start(out=bq_sb, in_=bq)
        bk_sb = consts.tile([P, DC], F32)
        nc.gpsimd.dma_start(out=bk_sb, in_=bk)
        bv_sb = consts.tile([1, DAUG], F32R)
        nc.gpsimd.dma_start(out=bv_sb, in_=bv)

        qT_sb = qkv.tile([P, DC, N], F16)
        kT_sb = qkv.tile([P, DC, N], F16)
        v_sb = qkv.tile([P, MC, DAUG], F32R)
        featsT_sb = qkv.tile([P, DC, N], F32R)

        # ---- QKV projections ----
        for w_tiles, b_sb, dst in ((wq_t, bq_sb, qT_sb), (wk_t, bk_sb, kT_sb)):
            for dc in range(DC):
                ps = sps.tile([P, N], F32, tag="sps")
                for nb in range(NB):
                    for hc in range(HC):
                        nc.tensor.matmul(
                            ps[:, nb * 512 : (nb + 1) * 512],
                            (w_tiles[hc][:, dc * P : (dc + 1) * P]),
                            (xT_t[hc][:, nb * 512 : (nb + 1) * 512]),
                            start=(hc == 0),
                            stop=(hc == HC - 1),
                        )
                nc.scalar.activation(
                    out=dst[:, dc, :],
                    in_=ps,
                    func=Act.Identity,
                    bias=b_sb[:, dc : dc + 1],
                )

        HF = DAUG // 2  # 260
        for nt in range(NT):
            ps = sps.tile([P, N], F32, tag="sps")
            for half in range(2):
                pv = ps[:, half * 512 : half * 512 + HF]
                for hc in range(HC):
                    nc.tensor.matmul(
                        pv,
                        (xT_t[hc][:, nt * P : (nt + 1) * P]),
                        (wv_t[hc][:, half * HF : (half + 1) * HF]),
                        start=(hc == 0),
                        stop=False,
                    )
                # bias row: v[n, :] += bv (also writes the ones columns)
                nc.tensor.matmul(
                    pv,
                    (ones_t[:1, :P]),
                    (bv_sb[:1, half * HF : (half + 1) * HF]),
                    start=False,
                    stop=True,
                )
            nc.scalar.copy(
                v_sb[:, nt, :].rearrange("p (h x) -> p h x", h=2),
                ps.rearrange("p (h x) -> p h x", h=2)[:, :, :HF],
            )

        stage1_ctx.close()

        # ---- attention, head pairs (row-packed scores) ----
        # Per (head, m-chunk): psum[m, n] = I @ biasT (start) + kT.T q (accum),
        # so the bias add runs on the PE and ACT exps straight out of PSUM.
        for hp in range(4):
            heads = (2 * hp, 2 * hp + 1)
            bias_t = {}

            def load_bias(h, j):
                bt = bias_pool.tile([P, 2, N], F16, tag="bias", name=f"bt{h}_{j}")
                nc.gpsimd.dma_start(
                    out=bt,
                    in_=biasT[h].rearrange("(c p) n -> p c n", p=P)[
                        :, 2 * j : 2 * j + 2
                    ],
                )
                bias_t[h] = bt

            f_ps = {
                (h, nb): fps.tile([HS + 1, 512], F32, tag="fps", name=f"fps{h}_{nb}")
                for h in heads
                for nb in range(NB)
            }
            for mc in range(MC):
                if mc % 2 == 0:
                    for h in heads:
                        load_bias(h, mc // 2)
                sp = {}
                et = {}
                for h in heads:
                    sp[h] = sps.tile([P, N], F32, tag="sps", name=f"sp{h}_{mc}")
                    et[h] = et_pool.tile([P, N], F32R, tag="et", name=f"et{h}_{mc}")
                    for nb in range(NB):
                        nc.tensor.matmul(
                            sp[h][:, nb * 512 : (nb + 1) * 512],
                            ident[:, :],
                            bias_t[h][:, mc % 2, nb * 512 : (nb + 1) * 512],
                            start=True,
                            stop=False,
                        )
                for nb in range(NB):
                    for h in heads:  # adjacent K=64 matmuls -> row-packed pair
                        dpo = (h % 2) * HS
                        nc.tensor.matmul(
                            sp[h][:, nb * 512 : (nb + 1) * 512],
                            kT_sb[dpo : dpo + HS, hp, mc * P : (mc + 1) * P],
                            qT_sb[dpo : dpo + HS, hp, nb * 512 : (nb + 1) * 512],
                            start=False,
                            stop=True,
                        )
                for h in heads:
                    nc.scalar.activation(out=et[h], in_=sp[h], func=Act.Exp)
                    for nb in range(NB):
                        nc.tensor.matmul(
                            f_ps[(h, nb)],
                            v_sb[:, mc, (HS + 1) * h : (HS + 1) * (h + 1)],
                            et[h][:, nb * 512 : (nb + 1) * 512],
                            start=(mc == 0),
                            stop=(mc == MC - 1),
                        )
            # normalize by the softmax denominator (psum row 64) and store
            for h in heads:
                po, ddc = HS * (h % 2), h // 2
                for nb in range(NB):
                    inv_s = small.tile([1, 512], F32, tag="inv", name=f"inv{h}{nb}")
                    nc.vector.reciprocal(inv_s, f_ps[(h, nb)][HS : HS + 1, :])
                    scr = inv_scr[2 * (h % 2) + nb : 2 * (h % 2) + nb + 1, :]
                    nc.gpsimd.dma_start(out=scr, in_=inv_s)
                    b_sb = small.tile([HS, 512], F32, tag="bcast", name=f"bc{h}{nb}")
                    nc.gpsimd.dma_start(out=b_sb, in_=scr.to_broadcast((HS, 512)))
                    nc.vector.tensor_mul(
                        out=featsT_sb[po : po + HS, ddc, nb * 512 : (nb + 1) * 512],
                        in0=f_ps[(h, nb)][:HS, :],
                        in1=b_sb,
                    )

        # ---- output projection (partial: contracts this core's 512 rows) ----
        for nt in range(NT):
            ps = sps.tile([P, N], F32, tag="sps")
            for cb in range(NB):
                for dc in range(DC):
                    nc.tensor.matmul(
                        ps[:, cb * 512 : (cb + 1) * 512],
                        (featsT_sb[:, dc, nt * P : (nt + 1) * P]),
                        (wp_sb[:, dc, cb * 512 : (cb + 1) * 512]),
                        start=(dc == 0),
                        stop=(dc == DC - 1),
                    )
            o_t = opool.tile([P, N], F32, tag="o")
            nc.scalar.copy(o_t, ps)
            nc.sync.dma_start(out=out[nt * P : (nt + 1) * P, :], in_=o_t)


def build_program():
    nc = bacc.Bacc("TRN2", target_bir_lowering=False, debug=False, num_devices=8)
    xT = nc.dram_tensor("xT", [H, N], F32R, kind="ExternalInput").ap()
    biasT = nc.dram_tensor("biasT", [HL, N, N], F16, kind="ExternalInput").ap()
    wq = nc.dram_tensor("wq", [H, DLOC], F32R, kind="ExternalInput").ap()
    wk = nc.dram_tensor("wk", [H, DLOC], F32R, kind="ExternalInput").ap()
    wv = nc.dram_tensor("wv", [H, DAUG], F32R, kind="ExternalInput").ap()
    wp = nc.dram_tensor("wp", [DLOC, H], F32R, kind="ExternalInput").ap()
    bq = nc.dram_tensor("bq", [P, DC], F32, kind="ExternalInput").ap()
    bk = nc.dram_tensor("bk", [P, DC], F32, kind="ExternalInput").ap()
    bv = nc.dram_tensor("bv", [1, DAUG], F32R, kind="ExternalInput").ap()
    ones = nc.dram_tensor("ones", [1, P], F32R, kind="ExternalInput").ap()
    ident_d = nc.dram_tensor("ident", [P, P], F16, kind="ExternalInput").ap()
    out = nc.dram_tensor("out", [N, H], F32, kind="ExternalOutput").ap()
    with tile.TileContext(nc) as tc:
        _emit(nc, tc, (xT, biasT, wq, wk, wv, wp, bq, bk, bv, ones, ident_d, out))
    nc.compile()
    return nc


def get_program():
    global _PROG
    if _PROG is None:
        _PROG = build_program()
    return _PROG


def make_in_maps(x, attn_bias, Wq, bq, Wk, bk, Wv, bv, Wp):
    """Host-side sharding: slice/transpose/augment per-core inputs."""
    f = np.float32
    x = np.asarray(x, f)
    attn_bias = np.asarray(attn_bias, f)
    wq_s = np.asarray(Wq, f) * f(SCALE)
    bq_s = np.asarray(bq, f) * f(SCALE)
    Wk, bk = np.asarray(Wk, f), np.asarray(bk, f)
    Wv, bv = np.asarray(Wv, f), np.asarray(bv, f)
    Wp = np.asarray(Wp, f)

    xTs = [np.ascontiguousarray(x[b].T) for b in range(B)]
    in_maps = []
    for c in range(8):
        b, g = divmod(c, 2)
        dsl = slice(DLOC * g, DLOC * (g + 1))
        wv_aug = np.zeros((H, DAUG), f)
        bv_aug = np.zeros((1, DAUG), f)
        for hl in range(HL):
            src = slice(DLOC * g + HS * hl, DLOC * g + HS * (hl + 1))
            dst = slice((HS + 1) * hl, (HS + 1) * hl + HS)
            wv_aug[:, dst] = Wv[:, src]
            bv_aug[0, dst] = bv[src]
            bv_aug[0, (HS + 1) * hl + HS] = 1.0
        in_maps.append(
            {
                "xT": xTs[b],
                "biasT": np.ascontiguousarray(
                    attn_bias[b, HL * g : HL * (g + 1)].transpose(0, 2, 1)
                ).astype(np.float16),
                "wq": np.ascontiguousarray(wq_s[:, dsl]),
                "wk": np.ascontiguousarray(Wk[:, dsl]),
                "wv": wv_aug,
                "wp": np.ascontiguousarray(Wp[dsl, :]),
                "bq": np.ascontiguousarray(bq_s[dsl].reshape(DC, P).T),
                "bk": np.ascontiguousarray(bk[dsl].reshape(DC, P).T),
                "bv": bv_aug,
                "ones": np.ones((1, P), f),
                "ident": np.eye(P, dtype=np.float16),
            }
        )
    return in_maps


def _ensure_ntff_hook():
    """Register the axon NTFF profile hook if the image's antenv lacks it."""
    try:
        from antenv.axon_hooks import get_axon_ntff_profile_hook  # noqa: F401

        return
    except ImportError:
        pass
    import types

    import antenv
    from trn_agent_boot.trn_boot import _ntff_profile_via_ctypes

    mod = types.ModuleType("antenv.axon_hooks")
    box = {"h": None}
    mod.set_axon_ntff_profile_hook = lambda h: box.__setitem__("h", h)
    mod.get_axon_ntff_profile_hook = lambda: box["h"]
    sys.modules["antenv.axon_hooks"] = mod
    antenv.axon_hooks = mod
    hook = _ntff_profile_via_ctypes("/opt/axon/libaxon_pjrt.so")
    if hook is not None:
        mod.set_axon_ntff_profile_hook(hook)


def run_cores(in_maps, trace=False):
    nc = get_program()
    kwargs = {}
    if trace:
        _ensure_ntff_hook()
        kwargs = dict(trace=True, trace_cores=[0])
    return run_bass_kernel_spmd(nc, in_maps, core_ids=list(range(8)), **kwargs)


def kernel(x, attn_bias, Wq, bq, Wk, bk, Wv, bv, Wp, bp):
    in_maps = make_in_maps(x, attn_bias, Wq, bq, Wk, bk, Wv, bv, Wp)
    res = run_cores(in_maps)
    bp = np.asarray(bp, np.float32)
    out = np.empty((B, N, H), np.float32)
    for b in range(B):
        out[b] = res.results[2 * b]["out"] + res.results[2 * b + 1]["out"] + bp
    return out


# revision 14
# speedup vs baseline: 1.2579x; 1.0528x over previous
"""Biased multi-head self-attention (B=4, N=1024, H=1024, 16 heads) on 8
Trainium2 NeuronCores.

Sharding: data-parallel over batch (4) x tensor-parallel over head-groups
(2 groups of 8 heads) = 8 cores. Core c handles batch c//2, head-group c%2.
Each core computes QKV projections for its 512 feature columns, biased
softmax attention for its 8 heads, and a partial output projection
(contracting its 512 feature rows of Wp). The two head-groups' partial
projections per batch are summed on the host (the "projection all-reduce"),
which also adds the output bias bp.

Device dataflow (per core); all matmuls run as fp32r (full-rate PE mode):
  - xT [h, n] (host-transposed x) and weights [h, d] feed the PE directly:
    qT/kT land in [d, n] layout, v in [n, d] layout.
  - The attention-score scale 1/sqrt(64) is folded into Wq/bq on the host.
  - Scores are computed TRANSPOSED, sT[m, n] (lhsT=kT, rhs=qT, K=64),
    because the PV matmul contracts over m, which must live on partitions;
    attn_bias is pre-transposed on the host for the same reason.
  - eT = exp(sT + biasT): DVE add (psum + sbuf), ACT exp in place.
  - v is stored with an interleaved all-ones column per head (via an
    augmented Wv/bv), so each head's PV matmul yields [65, n]: rows 0..63
    are unnormalized feats^T, row 64 is the softmax denominator s[n].
  - Normalization multiplies by 1/s broadcast across partitions via a K=1
    ones matmul.
  - out_partial [n, 1024] = featsT.T @ Wp_slice.
"""

import sys

for _p in ("/opt/trn_rl_repo", "/opt/pypackages"):
    if _p not in sys.path:
        sys.path.append(_p)

import numpy as np

import concourse.bass as bass
import concourse.bacc as bacc
import concourse.mybir as mybir
import concourse.tile as tile
from concourse.bass_utils import run_bass_kernel_spmd

P = 128
N = 1024          # sequence length
H = 1024          # model dim
B = 4
NH = 16
HS = 64
G = 2             # head groups (tensor parallel)
HL = NH // G      # heads per core = 8
DLOC = H // G     # feature cols per core = 512
DAUG = HL * (HS + 1)   # 520: v with interleaved ones column
HC = H // P       # 8 contraction chunks over model dim
DC = DLOC // P    # 4 chunks over local feature dim
NB = N // 512     # 2 moving-dim blocks
NT = N // P       # 8 n tiles
MC = N // P       # 8 m chunks
SCALE = 1.0 / np.sqrt(HS)

F32 = mybir.dt.float32
F32R = mybir.dt.float32r
F16 = mybir.dt.float16
Act = mybir.ActivationFunctionType

_PROG = None


def _emit(nc, tc, io):
    xT, biasT, wq, wk, wv, wp, bq, bk, bv, ones, ident_d, out = io

    import contextlib

    with contextlib.ExitStack() as ctx:
        consts = ctx.enter_context(tc.tile_pool(name="consts", bufs=1))
        qkv = ctx.enter_context(tc.tile_pool(name="qkv", bufs=1))
        opool = ctx.enter_context(tc.tile_pool(name="opool", bufs=2))
        small = ctx.enter_context(tc.tile_pool(name="small", bufs=3))
        sps = ctx.enter_context(tc.tile_pool(name="sps", bufs=2, space="PSUM"))
        fps = ctx.enter_context(tc.tile_pool(name="fps", bufs=4, space="PSUM"))
        bias_pool = ctx.enter_context(tc.tile_pool(name="bias", bufs=8))
        et_pool = ctx.enter_context(tc.tile_pool(name="et", bufs=3))
        stage1_ctx = contextlib.ExitStack()
        stage1 = stage1_ctx.enter_context(tc.tile_pool(name="stage1", bufs=1))

        ones_t = consts.tile([1, P], F16)
        nc.gpsimd.dma_start(out=ones_t, in_=ones)
        inv_scr = nc.dram_tensor("inv_scr", [4, 512], F32).ap()
        ident = consts.tile([P, P], F16)
        nc.sync.dma_start(out=ident, in_=ident_d)

        xT_r = xT.rearrange("(c p) n -> p c n", p=P)
        wq_r = wq.rearrange("(c p) d -> p c d", p=P)
        wk_r = wk.rearrange("(c p) d -> p c d", p=P)
        xT_t, wq_t, wk_t, wv_t = [], [], [], []
        for hc in range(HC):
            xt = stage1.tile([P, N], F16, name=f"xt{hc}")
            nc.sync.dma_start(out=xt, in_=xT_r[:, hc])
            xT_t.append(xt)
            wt = stage1.tile([P, DLOC], F16, name=f"wq{hc}")
            nc.sync.dma_start(out=wt, in_=wq_r[:, hc])
            wq_t.append(wt)
        for hc in range(HC):
            wt = stage1.tile([P, DLOC], F16, name=f"wk{hc}")
            nc.sync.dma_start(out=wt, in_=wk_r[:, hc])
            wk_t.append(wt)
        wv_r = wv.rearrange("(c p) d -> p c d", p=P)
        for hc in range(HC):
            wt = stage1.tile([P, DAUG], F16, name=f"wv{hc}")
            nc.sync.dma_start(out=wt, in_=wv_r[:, hc])
            wv_t.append(wt)

        wp_sb = consts.tile([P, DC, H], F16)
        nc.sync.dma_start(out=wp_sb, in_=wp.rearrange("(c p) o -> p c o", p=P))
        bq_sb = consts.tile([P, DC], F32)
        nc.gpsimd.dma_start(out=bq_sb, in_=bq)
        bk_sb = consts.tile([P, DC], F32)
        nc.gpsimd.dma_start(out=bk_sb, in_=bk)
        bv_sb = consts.tile([1, DAUG], F16)
        nc.gpsimd.dma_start(out=bv_sb, in_=bv)

        qT_sb = qkv.tile([P, DC, N], F16)
        kT_sb = qkv.tile([P, DC, N], F16)
        v_sb = qkv.tile([P, MC, DAUG], F16)
        featsT_sb = qkv.tile([P, DC, N], F16)

        # ---- QKV projections ----
        for w_tiles, b_sb, dst in ((wq_t, bq_sb, qT_sb), (wk_t, bk_sb, kT_sb)):
            for dc in range(DC):
                ps = sps.tile([P, N], F32, tag="sps")
                for nb in range(NB):
                    for hc in range(HC):
                        nc.tensor.matmul(
                            ps[:, nb * 512 : (nb + 1) * 512],
                            (w_tiles[hc][:, dc * P : (dc + 1) * P]),
                            (xT_t[hc][:, nb * 512 : (nb + 1) * 512]),
                            start=(hc == 0),
                            stop=(hc == HC - 1),
                        )
                nc.scalar.activation(
                    out=dst[:, dc, :],
                    in_=ps,
                    func=Act.Identity,
                    bias=b_sb[:, dc : dc + 1],
                )

        HF = DAUG // 2  # 260
        for nt in range(NT):
            ps = sps.tile([P, N], F32, tag="sps")
            for half in range(2):
                pv = ps[:, half * 512 : half * 512 + HF]
                for hc in range(HC):
                    nc.tensor.matmul(
                        pv,
                        (xT_t[hc][:, nt * P : (nt + 1) * P]),
                        (wv_t[hc][:, half * HF : (half + 1) * HF]),
                        start=(hc == 0),
                        stop=False,
                    )
                # bias row: v[n, :] += bv (also writes the ones columns)
                nc.tensor.matmul(
                    pv,
                    (ones_t[:1, :P]),
                    (bv_sb[:1, half * HF : (half + 1) * HF]),
                    start=False,
                    stop=True,
                )
            nc.scalar.copy(
                v_sb[:, nt, :].rearrange("p (h x) -> p h x", h=2),
                ps.rearrange("p (h x) -> p h x", h=2)[:, :, :HF],
            )

        stage1_ctx.close()

        # ---- attention, head pairs (row-packed scores) ----
        # Per (head, m-chunk): psum[m, n] = I @ biasT (start) + kT.T q (accum),
        # so the bias add runs on the PE and ACT exps straight out of PSUM.
        for hp in range(4):
            heads = (2 * hp, 2 * hp + 1)
            bias_t = {}

            def load_bias(h, j):
                bt = bias_pool.tile([P, 2, N], F16, tag="bias", name=f"bt{h}_{j}")
                nc.gpsimd.dma_start(
                    out=bt,
                    in_=biasT[h].rearrange("(c p) n -> p c n", p=P)[
                        :, 2 * j : 2 * j + 2
                    ],
                )
                bias_t[h] = bt

            f_ps = {
                (h, nb): fps.tile([HS + 1, 512], F32, tag="fps", name=f"fps{h}_{nb}")
                for h in heads
                for nb in range(NB)
            }
            for mc in range(MC):
                if mc % 2 == 0:
                    for h in heads:
                        load_bias(h, mc // 2)
                sp = {}
                et = {}
                for h in heads:
                    sp[h] = sps.tile([P, N], F32, tag="sps", name=f"sp{h}_{mc}")
                    et[h] = et_pool.tile([P, N], F16, tag="et", name=f"et{h}_{mc}")
                    for nb in range(NB):
                        nc.tensor.matmul(
                            sp[h][:, nb * 512 : (nb + 1) * 512],
                            ident[:, :],
                            bias_t[h][:, mc % 2, nb * 512 : (nb + 1) * 512],
                            start=True,
                            stop=False,
                        )
                for nb in range(NB):
                    for h in heads:  # adjacent K=64 matmuls -> row-packed pair
                        dpo = (h % 2) * HS
                        nc.tensor.matmul(
                            sp[h][:, nb * 512 : (nb + 1) * 512],
                            kT_sb[dpo : dpo + HS, hp, mc * P : (mc + 1) * P],
                            qT_sb[dpo : dpo + HS, hp, nb * 512 : (nb + 1) * 512],
                            start=False,
                            stop=True,
                        )
                for h in heads:
                    nc.scalar.activation(out=et[h], in_=sp[h], func=Act.Exp)
                    for nb in range(NB):
                        nc.tensor.matmul(
                            f_ps[(h, nb)],
                            v_sb[:, mc, (HS + 1) * h : (HS + 1) * (h + 1)],
                            et[h][:, nb * 512 : (nb + 1) * 512],
                            start=(mc == 0),
                            stop=(mc == MC - 1),
                        )
            # normalize by the softmax denominator (psum row 64) and store
            for h in heads:
                po, ddc = HS * (h % 2), h // 2
                for nb in range(NB):
                    inv_s = small.tile([1, 512], F32, tag="inv", name=f"inv{h}{nb}")
                    nc.vector.reciprocal(inv_s, f_ps[(h, nb)][HS : HS + 1, :])
                    scr = inv_scr[2 * (h % 2) + nb : 2 * (h % 2) + nb + 1, :]
                    nc.gpsimd.dma_start(out=scr, in_=inv_s)
                    b_sb = small.tile([HS, 512], F32, tag="bcast", name=f"bc{h}{nb}")
                    nc.gpsimd.dma_start(out=b_sb, in_=scr.to_broadcast((HS, 512)))
                    nc.vector.tensor_mul(
                        out=featsT_sb[po : po + HS, ddc, nb * 512 : (nb + 1) * 512],
                        in0=f_ps[(h, nb)][:HS, :],
                        in1=b_sb,
                    )

        # ---- output projection (partial: contracts this core's 512 rows) ----
        for nt in range(NT):
            ps = sps.tile([P, N], F32, tag="sps")
            for cb in range(NB):
                for dc in range(DC):
                    nc.tensor.matmul(
                        ps[:, cb * 512 : (cb + 1) * 512],
                        (featsT_sb[:, dc, nt * P : (nt + 1) * P]),
                        (wp_sb[:, dc, cb * 512 : (cb + 1) * 512]),
                        start=(dc == 0),
                        stop=(dc == DC - 1),
                    )
            o_t = opool.tile([P, N], F32, tag="o")
            nc.scalar.copy(o_t, ps)
            nc.sync.dma_start(out=out[nt * P : (nt + 1) * P, :], in_=o_t)


def build_program():
    nc = bacc.Bacc("TRN2", target_bir_lowering=False, debug=False, num_devices=8)
    xT = nc.dram_tensor("xT", [H, N], F16, kind="ExternalInput").ap()
    biasT = nc.dram_tensor("biasT", [HL, N, N], F16, kind="ExternalInput").ap()
    wq = nc.dram_tensor("wq", [H, DLOC], F16, kind="ExternalInput").ap()
    wk = nc.dram_tensor("wk", [H, DLOC], F16, kind="ExternalInput").ap()
    wv = nc.dram_tensor("wv", [H, DAUG], F16, kind="ExternalInput").ap()
    wp = nc.dram_tensor("wp", [DLOC, H], F16, kind="ExternalInput").ap()
    bq = nc.dram_tensor("bq", [P, DC], F32, kind="ExternalInput").ap()
    bk = nc.dram_tensor("bk", [P, DC], F32, kind="ExternalInput").ap()
    bv = nc.dram_tensor("bv", [1, DAUG], F16, kind="ExternalInput").ap()
    ones = nc.dram_tensor("ones", [1, P], F16, kind="ExternalInput").ap()
    ident_d = nc.dram_tensor("ident", [P, P], F16, kind="ExternalInput").ap()
    out = nc.dram_tensor("out", [N, H], F32, kind="ExternalOutput").ap()
    with tile.TileContext(nc) as tc:
        _emit(nc, tc, (xT, biasT, wq, wk, wv, wp, bq, bk, bv, ones, ident_d, out))
    nc.compile()
    return nc


def get_program():
    global _PROG
    if _PROG is None:
        _PROG = build_program()
    return _PROG


def make_in_maps(x, attn_bias, Wq, bq, Wk, bk, Wv, bv, Wp):
    """Host-side sharding: slice/transpose/augment per-core inputs."""
    f = np.float32
    x = np.asarray(x, f)
    attn_bias = np.asarray(attn_bias, f)
    wq_s = np.asarray(Wq, f) * f(SCALE)
    bq_s = np.asarray(bq, f) * f(SCALE)
    Wk, bk = np.asarray(Wk, f), np.asarray(bk, f)
    Wv, bv = np.asarray(Wv, f), np.asarray(bv, f)
    Wp = np.asarray(Wp, f)

    xTs = [np.ascontiguousarray(x[b].T).astype(np.float16) for b in range(B)]
    in_maps = []
    for c in range(8):
        b, g = divmod(c, 2)
        dsl = slice(DLOC * g, DLOC * (g + 1))
        wv_aug = np.zeros((H, DAUG), np.float16)
        bv_aug = np.zeros((1, DAUG), np.float16)
        for hl in range(HL):
            src = slice(DLOC * g + HS * hl, DLOC * g + HS * (hl + 1))
            dst = slice((HS + 1) * hl, (HS + 1) * hl + HS)
            wv_aug[:, dst] = Wv[:, src]
            bv_aug[0, dst] = bv[src]
            bv_aug[0, (HS + 1) * hl + HS] = 1.0
        in_maps.append(
            {
                "xT": xTs[b],
                "biasT": np.ascontiguousarray(
                    attn_bias[b, HL * g : HL * (g + 1)].transpose(0, 2, 1)
                ).astype(np.float16),
                "wq": np.ascontiguousarray(wq_s[:, dsl]).astype(np.float16),
                "wk": np.ascontiguousarray(Wk[:, dsl]).astype(np.float16),
                "wv": wv_aug,
                "wp": np.ascontiguousarray(Wp[dsl, :]).astype(np.float16),
                "bq": np.ascontiguousarray(bq_s[dsl].reshape(DC, P).T),
                "bk": np.ascontiguousarray(bk[dsl].reshape(DC, P).T),
                "bv": bv_aug,
                "ones": np.ones((1, P), np.float16),
                "ident": np.eye(P, dtype=np.float16),
            }
        )
    return in_maps


def _ensure_ntff_hook():
    """Register the axon NTFF profile hook if the image's antenv lacks it."""
    try:
        from antenv.axon_hooks import get_axon_ntff_profile_hook  # noqa: F401

        return
    except ImportError:
        pass
    import types

    import antenv
    from trn_agent_boot.trn_boot import _ntff_profile_via_ctypes

    mod = types.ModuleType("antenv.axon_hooks")
    box = {"h": None}
    mod.set_axon_ntff_profile_hook = lambda h: box.__setitem__("h", h)
    mod.get_axon_ntff_profile_hook = lambda: box["h"]
    sys.modules["antenv.axon_hooks"] = mod
    antenv.axon_hooks = mod
    hook = _ntff_profile_via_ctypes("/opt/axon/libaxon_pjrt.so")
    if hook is not None:
        mod.set_axon_ntff_profile_hook(hook)


def run_cores(in_maps, trace=False):
    nc = get_program()
    kwargs = {}
    if trace:
        _ensure_ntff_hook()
        kwargs = dict(trace=True, trace_cores=[0])
    return run_bass_kernel_spmd(nc, in_maps, core_ids=list(range(8)), **kwargs)


def kernel(x, attn_bias, Wq, bq, Wk, bk, Wv, bv, Wp, bp):
    in_maps = make_in_maps(x, attn_bias, Wq, bq, Wk, bk, Wv, bv, Wp)
    res = run_cores(in_maps)
    bp = np.asarray(bp, np.float32)
    out = np.empty((B, N, H), np.float32)
    for b in range(B):
        out[b] = res.results[2 * b]["out"] + res.results[2 * b + 1]["out"] + bp
    return out


# revision 15
# speedup vs baseline: 1.3670x; 1.0868x over previous
"""Biased multi-head self-attention (B=4, N=1024, H=1024, 16 heads) on 8
Trainium2 NeuronCores.

Sharding: data-parallel over batch (4) x tensor-parallel over head-groups
(2 groups of 8 heads) = 8 cores. Core c handles batch c//2, head-group c%2.
Each core computes QKV projections for its 512 feature columns, biased
softmax attention for its 8 heads, and a partial output projection
(contracting its 512 feature rows of Wp). The two head-groups' partial
projections per batch are summed on the host (the "projection all-reduce"),
which also adds the output bias bp.

Device dataflow (per core); all matmuls run as fp32r (full-rate PE mode):
  - xT [h, n] (host-transposed x) and weights [h, d] feed the PE directly:
    qT/kT land in [d, n] layout, v in [n, d] layout.
  - The attention-score scale 1/sqrt(64) is folded into Wq/bq on the host.
  - Scores are computed TRANSPOSED, sT[m, n] (lhsT=kT, rhs=qT, K=64),
    because the PV matmul contracts over m, which must live on partitions;
    attn_bias is pre-transposed on the host for the same reason.
  - eT = exp(sT + biasT): DVE add (psum + sbuf), ACT exp in place.
  - v is stored with an interleaved all-ones column per head (via an
    augmented Wv/bv), so each head's PV matmul yields [65, n]: rows 0..63
    are unnormalized feats^T, row 64 is the softmax denominator s[n].
  - Normalization multiplies by 1/s broadcast across partitions via a K=1
    ones matmul.
  - out_partial [n, 1024] = featsT.T @ Wp_slice.
"""

import sys

for _p in ("/opt/trn_rl_repo", "/opt/pypackages"):
    if _p not in sys.path:
        sys.path.append(_p)

import numpy as np

import concourse.bass as bass
import concourse.bacc as bacc
import concourse.mybir as mybir
import concourse.tile as tile
from concourse.bass_utils import run_bass_kernel_spmd

P = 128
N = 1024          # sequence length
H = 1024          # model dim
B = 4
NH = 16
HS = 64
G = 2             # head groups (tensor parallel)
HL = NH // G      # heads per core = 8
DLOC = H // G     # feature cols per core = 512
DAUG = HL * (HS + 1)   # 520: v with interleaved ones column
HC = H // P       # 8 contraction chunks over model dim
DC = DLOC // P    # 4 chunks over local feature dim
NB = N // 512     # 2 moving-dim blocks
NT = N // P       # 8 n tiles
MC = N // P       # 8 m chunks
SCALE = 1.0 / np.sqrt(HS)

F32 = mybir.dt.float32
F32R = mybir.dt.float32r
F16 = mybir.dt.float16
Act = mybir.ActivationFunctionType

_PROG = None


def _emit(nc, tc, io):
    xT, biasT, wq, wk, wv, wp, bq, bk, bv, ones, ident_d, out = io

    import contextlib

    with contextlib.ExitStack() as ctx:
        consts = ctx.enter_context(tc.tile_pool(name="consts", bufs=1))
        qkv = ctx.enter_context(tc.tile_pool(name="qkv", bufs=1))
        opool = ctx.enter_context(tc.tile_pool(name="opool", bufs=2))
        small = ctx.enter_context(tc.tile_pool(name="small", bufs=3))
        sps = ctx.enter_context(tc.tile_pool(name="sps", bufs=2, space="PSUM"))
        fps = ctx.enter_context(tc.tile_pool(name="fps", bufs=4, space="PSUM"))
        bias_pool = ctx.enter_context(tc.tile_pool(name="bias", bufs=8))
        et_pool = ctx.enter_context(tc.tile_pool(name="et", bufs=3))
        fu_pool = ctx.enter_context(tc.tile_pool(name="fu", bufs=8))
        stage1_ctx = contextlib.ExitStack()
        stage1 = stage1_ctx.enter_context(tc.tile_pool(name="stage1", bufs=1))

        ones_t = consts.tile([1, P], F16)
        nc.gpsimd.dma_start(out=ones_t, in_=ones)
        inv_scr = nc.dram_tensor("inv_scr", [4, 512], F32).ap()
        ident = consts.tile([P, P], F16)
        nc.sync.dma_start(out=ident, in_=ident_d)

        xT_r = xT.rearrange("(c p) n -> p c n", p=P)
        wq_r = wq.rearrange("(c p) d -> p c d", p=P)
        wk_r = wk.rearrange("(c p) d -> p c d", p=P)
        xT_t, wq_t, wk_t, wv_t = [], [], [], []
        for hc in range(HC):
            xt = stage1.tile([P, N], F16, name=f"xt{hc}")
            nc.sync.dma_start(out=xt, in_=xT_r[:, hc])
            xT_t.append(xt)
            wt = stage1.tile([P, DLOC], F16, name=f"wq{hc}")
            nc.sync.dma_start(out=wt, in_=wq_r[:, hc])
            wq_t.append(wt)
        for hc in range(HC):
            wt = stage1.tile([P, DLOC], F16, name=f"wk{hc}")
            nc.sync.dma_start(out=wt, in_=wk_r[:, hc])
            wk_t.append(wt)
        wv_r = wv.rearrange("(c p) d -> p c d", p=P)
        for hc in range(HC):
            wt = stage1.tile([P, DAUG], F16, name=f"wv{hc}")
            nc.sync.dma_start(out=wt, in_=wv_r[:, hc])
            wv_t.append(wt)

        wp_sb = consts.tile([P, DC, H], F16)
        nc.sync.dma_start(out=wp_sb, in_=wp.rearrange("(c p) o -> p c o", p=P))
        bq_sb = consts.tile([P, DC], F32)
        nc.gpsimd.dma_start(out=bq_sb, in_=bq)
        bk_sb = consts.tile([P, DC], F32)
        nc.gpsimd.dma_start(out=bk_sb, in_=bk)
        bv_sb = consts.tile([1, DAUG], F16)
        nc.gpsimd.dma_start(out=bv_sb, in_=bv)

        qT_sb = qkv.tile([P, DC, N], F16)
        kT_sb = qkv.tile([P, DC, N], F16)
        v_sb = qkv.tile([P, MC, DAUG], F16)
        featsT_sb = qkv.tile([P, DC, N], F16)

        # ---- QKV projections ----
        for w_tiles, b_sb, dst in ((wq_t, bq_sb, qT_sb), (wk_t, bk_sb, kT_sb)):
            for dc in range(DC):
                ps = sps.tile([P, N], F32, tag="sps")
                for nb in range(NB):
                    for hc in range(HC):
                        nc.tensor.matmul(
                            ps[:, nb * 512 : (nb + 1) * 512],
                            (w_tiles[hc][:, dc * P : (dc + 1) * P]),
                            (xT_t[hc][:, nb * 512 : (nb + 1) * 512]),
                            start=(hc == 0),
                            stop=(hc == HC - 1),
                        )
                nc.scalar.activation(
                    out=dst[:, dc, :],
                    in_=ps,
                    func=Act.Identity,
                    bias=b_sb[:, dc : dc + 1],
                )

        HF = DAUG // 2  # 260
        for nt in range(NT):
            ps = sps.tile([P, N], F32, tag="sps")
            for half in range(2):
                pv = ps[:, half * 512 : half * 512 + HF]
                for hc in range(HC):
                    nc.tensor.matmul(
                        pv,
                        (xT_t[hc][:, nt * P : (nt + 1) * P]),
                        (wv_t[hc][:, half * HF : (half + 1) * HF]),
                        start=(hc == 0),
                        stop=False,
                    )
                # bias row: v[n, :] += bv (also writes the ones columns)
                nc.tensor.matmul(
                    pv,
                    (ones_t[:1, :P]),
                    (bv_sb[:1, half * HF : (half + 1) * HF]),
                    start=False,
                    stop=True,
                )
            nc.scalar.copy(
                v_sb[:, nt, :].rearrange("p (h x) -> p h x", h=2),
                ps.rearrange("p (h x) -> p h x", h=2)[:, :, :HF],
            )

        stage1_ctx.close()

        # ---- attention, head pairs (row-packed scores) ----
        # Per (head, m-chunk): psum[m, n] = I @ biasT (start) + kT.T q (accum),
        # so the bias add runs on the PE and ACT exps straight out of PSUM.
        for hp in range(4):
            heads = (2 * hp, 2 * hp + 1)
            bias_t = {}

            def load_bias(h, j):
                bt = bias_pool.tile([P, 2, N], F16, tag="bias", name=f"bt{h}_{j}")
                nc.gpsimd.dma_start(
                    out=bt,
                    in_=biasT[h].rearrange("(c p) n -> p c n", p=P)[
                        :, 2 * j : 2 * j + 2
                    ],
                )
                bias_t[h] = bt

            f_ps = {
                (h, nb): fps.tile([HS + 1, 512], F32, tag="fps", name=f"fps{h}_{nb}")
                for h in heads
                for nb in range(NB)
            }
            for mc in range(MC):
                if mc % 2 == 0:
                    for h in heads:
                        load_bias(h, mc // 2)
                sp = {}
                et = {}
                for h in heads:
                    sp[h] = sps.tile([P, N], F32, tag="sps", name=f"sp{h}_{mc}")
                    et[h] = et_pool.tile([P, N], F16, tag="et", name=f"et{h}_{mc}")
                    for nb in range(NB):
                        nc.tensor.matmul(
                            sp[h][:, nb * 512 : (nb + 1) * 512],
                            ident[:, :],
                            bias_t[h][:, mc % 2, nb * 512 : (nb + 1) * 512],
                            start=True,
                            stop=False,
                        )
                for nb in range(NB):
                    for h in heads:  # adjacent K=64 matmuls -> row-packed pair
                        dpo = (h % 2) * HS
                        nc.tensor.matmul(
                            sp[h][:, nb * 512 : (nb + 1) * 512],
                            kT_sb[dpo : dpo + HS, hp, mc * P : (mc + 1) * P],
                            qT_sb[dpo : dpo + HS, hp, nb * 512 : (nb + 1) * 512],
                            start=False,
                            stop=True,
                        )
                for h in heads:
                    nc.scalar.activation(out=et[h], in_=sp[h], func=Act.Exp)
                    for nb in range(NB):
                        nc.tensor.matmul(
                            f_ps[(h, nb)],
                            v_sb[:, mc, (HS + 1) * h : (HS + 1) * (h + 1)],
                            et[h][:, nb * 512 : (nb + 1) * 512],
                            start=(mc == 0),
                            stop=(mc == MC - 1),
                        )
            # evacuate feats psum fast (frees the banks for the next pair),
            # then normalize from SBUF off the PE critical path
            fu = {}
            for h in heads:
                for nb in range(NB):
                    t = fu_pool.tile([HS + 1, 512], F32, tag="fu", name=f"fu{h}{nb}")
                    nc.scalar.copy(t, f_ps[(h, nb)])
                    fu[(h, nb)] = t
            for h in heads:
                po, ddc = HS * (h % 2), h // 2
                for nb in range(NB):
                    inv_s = small.tile([1, 512], F32, tag="inv", name=f"inv{h}{nb}")
                    nc.vector.reciprocal(inv_s, fu[(h, nb)][HS : HS + 1, :])
                    scr = inv_scr[2 * (h % 2) + nb : 2 * (h % 2) + nb + 1, :]
                    nc.gpsimd.dma_start(out=scr, in_=inv_s)
                    b_sb = small.tile([HS, 512], F32, tag="bcast", name=f"bc{h}{nb}")
                    nc.gpsimd.dma_start(out=b_sb, in_=scr.to_broadcast((HS, 512)))
                    nc.vector.tensor_mul(
                        out=featsT_sb[po : po + HS, ddc, nb * 512 : (nb + 1) * 512],
                        in0=fu[(h, nb)][:HS, :],
                        in1=b_sb,
                    )

        # ---- output projection (partial: contracts this core's 512 rows) ----
        for nt in range(NT):
            ps = sps.tile([P, N], F32, tag="sps")
            for cb in range(NB):
                for dc in range(DC):
                    nc.tensor.matmul(
                        ps[:, cb * 512 : (cb + 1) * 512],
                        (featsT_sb[:, dc, nt * P : (nt + 1) * P]),
                        (wp_sb[:, dc, cb * 512 : (cb + 1) * 512]),
                        start=(dc == 0),
                        stop=(dc == DC - 1),
                    )
            o_t = opool.tile([P, N], F32, tag="o")
            nc.scalar.copy(o_t, ps)
            nc.sync.dma_start(out=out[nt * P : (nt + 1) * P, :], in_=o_t)


def build_program():
    nc = bacc.Bacc("TRN2", target_bir_lowering=False, debug=False, num_devices=8)
    xT = nc.dram_tensor("xT", [H, N], F16, kind="ExternalInput").ap()
    biasT = nc.dram_tensor("biasT", [HL, N, N], F16, kind="ExternalInput").ap()
    wq = nc.dram_tensor("wq", [H, DLOC], F16, kind="ExternalInput").ap()
    wk = nc.dram_tensor("wk", [H, DLOC], F16, kind="ExternalInput").ap()
    wv = nc.dram_tensor("wv", [H, DAUG], F16, kind="ExternalInput").ap()
    wp = nc.dram_tensor("wp", [DLOC, H], F16, kind="ExternalInput").ap()
    bq = nc.dram_tensor("bq", [P, DC], F32, kind="ExternalInput").ap()
    bk = nc.dram_tensor("bk", [P, DC], F32, kind="ExternalInput").ap()
    bv = nc.dram_tensor("bv", [1, DAUG], F16, kind="ExternalInput").ap()
    ones = nc.dram_tensor("ones", [1, P], F16, kind="ExternalInput").ap()
    ident_d = nc.dram_tensor("ident", [P, P], F16, kind="ExternalInput").ap()
    out = nc.dram_tensor("out", [N, H], F32, kind="ExternalOutput").ap()
    with tile.TileContext(nc) as tc:
        _emit(nc, tc, (xT, biasT, wq, wk, wv, wp, bq, bk, bv, ones, ident_d, out))
    nc.compile()
    return nc


def get_program():
    global _PROG
    if _PROG is None:
        _PROG = build_program()
    return _PROG


def make_in_maps(x, attn_bias, Wq, bq, Wk, bk, Wv, bv, Wp):
    """Host-side sharding: slice/transpose/augment per-core inputs."""
    f = np.float32
    x = np.asarray(x, f)
    attn_bias = np.asarray(attn_bias, f)
    wq_s = np.asarray(Wq, f) * f(SCALE)
    bq_s = np.asarray(bq, f) * f(SCALE)
    Wk, bk = np.asarray(Wk, f), np.asarray(bk, f)
    Wv, bv = np.asarray(Wv, f), np.asarray(bv, f)
    Wp = np.asarray(Wp, f)

    xTs = [np.ascontiguousarray(x[b].T).astype(np.float16) for b in range(B)]
    in_maps = []
    for c in range(8):
        b, g = divmod(c, 2)
        dsl = slice(DLOC * g, DLOC * (g + 1))
        wv_aug = np.zeros((H, DAUG), np.float16)
        bv_aug = np.zeros((1, DAUG), np.float16)
        for hl in range(HL):
            src = slice(DLOC * g + HS * hl, DLOC * g + HS * (hl + 1))
            dst = slice((HS + 1) * hl, (HS + 1) * hl + HS)
            wv_aug[:, dst] = Wv[:, src]
            bv_aug[0, dst] = bv[src]
            bv_aug[0, (HS + 1) * hl + HS] = 1.0
        in_maps.append(
            {
                "xT": xTs[b],
                "biasT": np.ascontiguousarray(
                    attn_bias[b, HL * g : HL * (g + 1)].transpose(0, 2, 1)
                ).astype(np.float16),
                "wq": np.ascontiguousarray(wq_s[:, dsl]).astype(np.float16),
                "wk": np.ascontiguousarray(Wk[:, dsl]).astype(np.float16),
                "wv": wv_aug,
                "wp": np.ascontiguousarray(Wp[dsl, :]).astype(np.float16),
                "bq": np.ascontiguousarray(bq_s[dsl].reshape(DC, P).T),
                "bk": np.ascontiguousarray(bk[dsl].reshape(DC, P).T),
                "bv": bv_aug,
                "ones": np.ones((1, P), np.float16),
                "ident": np.eye(P, dtype=np.float16),
            }
        )
    return in_maps


def _ensure_ntff_hook():
    """Register the axon NTFF profile hook if the image's antenv lacks it."""
    try:
        from antenv.axon_hooks import get_axon_ntff_profile_hook  # noqa: F401

        return
    except ImportError:
        pass
    import types

    import antenv
    from trn_agent_boot.trn_boot import _ntff_profile_via_ctypes

    mod = types.ModuleType("antenv.axon_hooks")
    box = {"h": None}
    mod.set_axon_ntff_profile_hook = lambda h: box.__setitem__("h", h)
    mod.get_axon_ntff_profile_hook = lambda: box["h"]
    sys.modules["antenv.axon_hooks"] = mod
    antenv.axon_hooks = mod
    hook = _ntff_profile_via_ctypes("/opt/axon/libaxon_pjrt.so")
    if hook is not None:
        mod.set_axon_ntff_profile_hook(hook)


def run_cores(in_maps, trace=False):
    nc = get_program()
    kwargs = {}
    if trace:
        _ensure_ntff_hook()
        kwargs = dict(trace=True, trace_cores=[0])
    return run_bass_kernel_spmd(nc, in_maps, core_ids=list(range(8)), **kwargs)


def kernel(x, attn_bias, Wq, bq, Wk, bk, Wv, bv, Wp, bp):
    in_maps = make_in_maps(x, attn_bias, Wq, bq, Wk, bk, Wv, bv, Wp)
    res = run_cores(in_maps)
    bp = np.asarray(bp, np.float32)
    out = np.empty((B, N, H), np.float32)
    for b in range(B):
        out[b] = res.results[2 * b]["out"] + res.results[2 * b + 1]["out"] + bp
    return out
